# revision 27
# baseline (speedup 1.0000x reference)
"""Trainium2 Bass kernel for nn_CCM_73985106641118 (vq_codebook).

Data-parallel across the batch dim: core b processes batch b (8 cores, B=8).

v2 design notes (vs the fp32 baseline):
- Every GEMM runs in fp32r (1 cyc/row at N>=512 on the PE vs 4 for fp32);
  empirically fp32r keeps ~1e-5 relative error, far inside the 2e-2 gate.
- Weights are DMA'd straight into single [128, 4*512] fp32r tiles (no
  staging copies through the vector engine).
- Hm is produced feature-major first (w2 chunks stationary) so the b2 bias
  lands in a scalar-engine drain; the token-major copy comes from PE
  transposes batched 4-at-a-time into one PSUM bank per drain.
- The C_pre l2-norms are computed via the Gram matrix  n2[h] =
  Ctemp[:,h]^T (M^T M) Ctemp[:,h]  so C_pre never needs to be
  materialized unscaled; Ctemp is pre-scaled once ([64,512]) and both the
  feature-major (H update) and token-major (C output) products come out
  normalized directly from the PE.
- PSUM drains are fused with the adjacent elementwise op wherever
  possible (relu+bias, bias add, +HmT, *theta, *invA) and spread across
  the scalar and vector engines.
"""

import numpy as np

import concourse.bacc as bacc
import concourse.mybir as mybir
from concourse.masks import make_identity
from concourse.tile import TileContext

f32 = mybir.dt.float32
f32r = mybir.dt.float32r
AX = mybir.AxisListType.X
OP = mybir.AluOpType
AF = mybir.ActivationFunctionType

B, N, C, H, K = 8, 2048, 512, 512, 64
NCP = N // 128   # 16 token chunks of 128
NCJ = N // 512   # 4 token chunks of 512
HC = H // 128    # 4 feature chunks of 128
SCALE = 1.0 / np.sqrt(np.float32(H))

_CACHE = {}


def s128(i):
    return slice(i * 128, (i + 1) * 128)


def s512(i):
    return slice(i * 512, (i + 1) * 512)


def s64(i):
    return slice(i * 64, (i + 1) * 64)


def build_nc(debug=False, upto=99):
    nc = bacc.Bacc("TRN2", target_bir_lowering=False, debug=False, num_devices=8)

    x_d = nc.dram_tensor("x", [N, C], f32, kind="ExternalInput").ap()
    bu_d = nc.dram_tensor("bern_u", [N, K], f32, kind="ExternalInput").ap()
    E_d = nc.dram_tensor("cluster_embeddings", [K, H], f32, kind="ExternalInput").ap()
    w1_d = nc.dram_tensor("mlp_w1", [C, H], f32, kind="ExternalInput").ap()
    b1_d = nc.dram_tensor("mlp_b1", [H], f32, kind="ExternalInput").ap()
    w2_d = nc.dram_tensor("mlp_w2", [H, H], f32, kind="ExternalInput").ap()
    b2_d = nc.dram_tensor("mlp_b2", [H], f32, kind="ExternalInput").ap()
    wq_d = nc.dram_tensor("wq", [H, H], f32, kind="ExternalInput").ap()
    wqb_d = nc.dram_tensor("wq_b", [H], f32, kind="ExternalInput").ap()
    wk_d = nc.dram_tensor("wk", [H, H], f32, kind="ExternalInput").ap()
    wkb_d = nc.dram_tensor("wk_b", [H], f32, kind="ExternalInput").ap()
    wout_d = nc.dram_tensor("wout", [H, C], f32, kind="ExternalInput").ap()
    woutb_d = nc.dram_tensor("wout_b", [C], f32, kind="ExternalInput").ap()
    Y_d = nc.dram_tensor("Y", [N, C], f32, kind="ExternalOutput").ap()
    Co_d = nc.dram_tensor("C_out", [N, H], f32, kind="ExternalOutput").ap()
    dbg = {}
    if debug:
        for nm, shp in [("P_dbg", [N, K]), ("M_dbg", [N, K]), ("HmT_dbg", [H, N]),
                        ("Ct_dbg", [64, H]), ("n2_dbg", [1, H])]:
            dbg[nm] = nc.dram_tensor(nm, shp, f32, kind="ExternalOutput").ap()

    with TileContext(nc) as tc:
        with (
            tc.tile_pool(name="big", bufs=4) as big,
            tc.tile_pool(name="med", bufs=1) as med,
            tc.tile_pool(name="sm", bufs=2) as sm,
            tc.tile_pool(name="stg", bufs=2) as stg,
            tc.tile_pool(name="psA", bufs=3, space="PSUM") as psA,
            tc.tile_pool(name="psT", bufs=2, space="PSUM") as psT,
            tc.tile_pool(name="psP", bufs=2, space="PSUM") as psP,
            tc.tile_pool(name="psS", bufs=1, space="PSUM") as psS,
        ):
            v = nc.vector
            sc = nc.scalar
            te = nc.tensor
            gp = nc.gpsimd

            # ---- x input DMAs first, on the Activation HWDGE queue, so
            # they don't queue behind the weight/embedding loads ------------
            xqs = []
            for j in range(NCJ):
                xq = stg.tile([128, N], f32, tag="xq", bufs=2, name=f"xq{j}")
                nc.scalar.dma_start(
                    out=xq[:].rearrange("p (q c) -> p q c", q=4),
                    in_=x_d[s512(j), :].rearrange("(q p) c -> p q c", p=128))
                xqs.append(xq)

            # ---- constants / weights -------------------------------------
            ident = med.tile([128, 128], f32, tag="ident")
            make_identity(nc, ident[:])

            def load_w(dram, tag):
                # 5 weights rotate through 3 slots: w1/w2 are dead after
                # phases 2/3, so wq/wout copies simply wait for those readers
                # (those two are loaded after the phase-2 code below so the
                # gpsimd queue order matches the dependency order).
                # fp32r matmul operands must be rounded by an engine op, so
                # stage each quarter and round via the (otherwise idle)
                # gpsimd engine.
                t = med.tile([128, HC * 512], f32r, tag="w", bufs=3, name=tag)
                for cc in range(HC):
                    st = sm.tile([128, 512], f32, tag="wst", bufs=2,
                                 name=f"wst_{tag}{cc}")
                    nc.sync.dma_start(out=st[:], in_=dram[s128(cc), :])
                    gp.tensor_copy(t[:, s512(cc)], st[:])
                return t

            w1t = load_w(w1_d, "w1t")
            w2t = load_w(w2_d, "w2t")
            wkt = load_w(wk_d, "wkt")

            # chunk (cc) stationary slice: rows cc*128.. of the [512, 512]
            # weight, columns block*128..
            def wchunk(t, cc, blk):
                return t[:, cc * 512 + blk * 128: cc * 512 + (blk + 1) * 128]

            E_f = med.tile([64, H], f32, tag="E")
            nc.sync.dma_start(out=E_f[:], in_=E_d[:, :])
            E_r = med.tile([64, H], f32r, tag="Er")
            gp.tensor_copy(E_r[:], E_f[:])

            def bias_cols(dram, tag):
                t = med.tile([128, HC], f32, tag=tag, name=tag)
                nc.sync.dma_start(out=t[:], in_=dram.rearrange("(j p) -> p j", p=128))
                return t

            b1c = bias_cols(b1_d, "b1c")
            b2c = bias_cols(b2_d, "b2c")
            wkbc = bias_cols(wkb_d, "wkbc")
            wqbc0 = bias_cols(wqb_d, "wqbc0")
            wqbc = med.tile([128, HC], f32, tag="wqbc")
            v.tensor_scalar(wqbc[:], wqbc0[:], float(SCALE), None, OP.mult)

            wobrow = med.tile([1, C], f32, tag="wobrow")
            nc.sync.dma_start(out=wobrow[:], in_=woutb_d.rearrange("(o a) -> o a", o=1))
            ones128 = med.tile([1, 128], f32, tag="ones")
            gp.memset(ones128[:], 1.0)
            ones64c = med.tile([64, 1], f32, tag="ones64c")
            gp.memset(ones64c[:], 1.0)
            ones1_64 = med.tile([1, 64], f32, tag="ones1_64")
            gp.memset(ones1_64[:], 1.0)
            # fp32r copies for the in-GEMM wout_b bias term
            ones128r = med.tile([1, 128], f32r, tag="onesr")
            gp.tensor_copy(ones128r[:], ones128[:])
            wobrow_r = med.tile([1, C], f32r, tag="wobrow_r")
            gp.tensor_copy(wobrow_r[:], wobrow[:])

            # E row norms -> Ebar (unit rows), EbarT / ET feature-major
            esq = sm.tile([128, C], f32, tag="xsq", bufs=1, name="esq")[0:64, :]
            ensq = med.tile([64, 1], f32, tag="ensq")
            sc.activation(esq[:], E_f[:], AF.Square, accum_out=ensq[:])
            enrm = med.tile([64, 1], f32, tag="enrm")
            sc.sqrt(enrm[:], ensq[:])
            einv = med.tile([64, 1], f32, tag="einv")
            v.reciprocal(einv[:], enrm[:])
            Ebar = med.tile([64, H], f32, tag="Ebar")
            v.tensor_scalar(Ebar[:], E_f[:], einv[:], None, OP.mult)

            EbarT, ET = [], []
            for hc in range(HC):
                pt = psP.tile([128, 64], f32, tag="small", bufs=2, name="ptE")
                te.transpose(pt[:], Ebar[:, s128(hc)], ident[0:64, 0:64])
                t = med.tile([128, 64], f32r, tag=f"ebt{hc}", name=f"ebt{hc}")
                v.tensor_copy(t[:], pt[:])
                EbarT.append(t)
                pt2 = psP.tile([128, 64], f32, tag="small", bufs=2, name="ptE2")
                te.transpose(pt2[:], E_f[:, s128(hc)], ident[0:64, 0:64])
                t2 = med.tile([128, 64], f32r, tag=f"et{hc}", name=f"et{hc}")
                v.tensor_copy(t2[:], pt2[:])
                ET.append(t2)

            # bern in one DMA: [128, 16*64], chunk ncp at cols ncp*64
            bern = med.tile([128, NCP * K], f32, tag="bern")
            nc.sync.dma_start(out=bern[:].rearrange("p (q k) -> p q k", q=16),
                              in_=bu_d.rearrange("(q p) k -> p q k", p=128))

            # ---- phase 1: xn = l2norm(x) rows (in place), xnT fp32r ------
            if upto >= 1:
                xnT = [big.tile([128, N], f32r, tag="A", name=f"xnT{i}")
                       for i in range(HC)]
                for j in range(NCJ):
                    xq = xqs[j]
                    ssq = sm.tile([128, 4], f32, tag="ssq", bufs=2, name="ssq")
                    xsq = sm.tile([128, C], f32, tag="xsq", bufs=1, name="xsq")
                    for q in range(4):
                        sc.activation(xsq[:], xq[:, s512(q)], AF.Square,
                                      accum_out=ssq[:, q:q + 1])
                    nrm = sm.tile([128, 4], f32, tag="nrm", bufs=2, name="nrm")
                    sc.sqrt(nrm[:], ssq[:])
                    nrm2 = sm.tile([128, 4], f32, tag="nrm2", bufs=2, name="nrm2")
                    v.tensor_scalar(nrm2[:], nrm[:], 1e-12, None, OP.max)
                    inv = sm.tile([128, 4], f32, tag="inv", bufs=2, name="inv")
                    v.reciprocal(inv[:], nrm2[:])
                    for q in range(4):
                        v.tensor_scalar(xq[:, s512(q)], xq[:, s512(q)],
                                        inv[:, q:q + 1], None, OP.mult)
                    for cc in range(HC):
                        pt = psT.tile([128, 512], f32, tag="tr", name="ptx")
                        for q in range(4):
                            te.transpose(pt[:, s128(q)],
                                         xq[:, q * 512 + cc * 128:
                                            q * 512 + (cc + 1) * 128],
                                         ident[:])
                        v.tensor_copy(xnT[cc][:, s512(j)], pt[:])

            # ---- phase 2: H1T = relu(w1.T @ xnT + b1)  fp32r -------------
            if upto >= 2:
                H1T = [big.tile([128, N], f32r, tag="B", name=f"H1T{i}")
                       for i in range(HC)]
                for j in range(NCJ):
                    for h1c in range(HC):
                        pp = psA.tile([128, 512], f32, tag="mm", name="ppH1")
                        for cc in range(HC):
                            te.matmul(pp[:], wchunk(w1t, cc, h1c),
                                      xnT[cc][:, s512(j)],
                                      start=(cc == 0), stop=(cc == HC - 1))
                        sc.activation(H1T[h1c][:, s512(j)], pp[:], AF.Relu,
                                      bias=b1c[:, h1c:h1c + 1], scale=1.0)

            # wq/wout reuse the w1/w2 slots; loading them here (after the
            # phase-2 trace) keeps the gpsimd queue deadlock-free.
            wqt = load_w(wq_d, "wqt")
            wot = load_w(wout_d, "wot")

            # ---- QT = (wq.T @ ET + wq_b) * scale -------------------------
            QT = []
            for hc in range(HC):
                pq = psP.tile([128, 64], f32, tag="small", bufs=2, name="pq")
                for cc in range(HC):
                    te.matmul(pq[:], wchunk(wqt, cc, hc), ET[cc][:],
                              start=(cc == 0), stop=(cc == HC - 1))
                t = med.tile([128, 64], f32r, tag=f"qt{hc}", name=f"qt{hc}")
                sc.activation(t[:], pq[:], AF.Identity,
                              bias=wqbc[:, hc:hc + 1], scale=float(SCALE))
                QT.append(t)

            # ---- phase 3: HmT = w2.T @ H1T + b2 (feature-major) ----------
            if upto >= 3:
                HmT = [big.tile([128, N], f32r, tag="C", name=f"HmT{i}")
                       for i in range(HC)]
                for j in range(NCJ):
                    for hc in range(HC):
                        pp = psA.tile([128, 512], f32, tag="mm", name="ppHm")
                        for h1c in range(HC):
                            te.matmul(pp[:], wchunk(w2t, h1c, hc),
                                      H1T[h1c][:, s512(j)],
                                      start=(h1c == 0), stop=(h1c == HC - 1))
                        v.tensor_scalar(HmT[hc][:, s512(j)], pp[:],
                                        b2c[:, hc:hc + 1], None, OP.add)
                        if debug:
                            nc.sync.dma_start(
                                out=dbg["HmT_dbg"][s128(hc), s512(j)],
                                in_=HmT[hc][:, s512(j)].bitcast(f32))

            # ---- phase 4: HmB token-major via PE transposes --------------
            if upto >= 4:
                HmB = [big.tile([128, N], f32r, tag="A", name=f"HmB{i}")
                       for i in range(NCJ)]
                for j in range(NCJ):
                    for q in range(4):
                        ncp = 4 * j + q
                        pt = psT.tile([128, 512], f32, tag="tr", name="ptm")
                        for hc in range(HC):
                            te.transpose(pt[:, s128(hc)],
                                         HmT[hc][:, s128(ncp)].bitcast(f32),
                                         ident[:])
                        sc.activation(HmB[j][:, s512(q)], pt[:], AF.Copy)

            # ---- phase 5: logits -> P -> M; PT / MT fp32r ----------------
            if upto >= 5:
                PT = med.tile([64, N], f32r, tag="PT")
                MT = med.tile([64, N], f32r, tag="MT")
                Mfull = med.tile([128, NCP * K], f32, tag="Mfull")
                for j in range(NCJ):
                    Pq = []
                    for q in range(4):
                        ncp = 4 * j + q
                        pl = psP.tile([128, 64], f32, tag="small", bufs=2,
                                      name="pl")
                        for hc in range(HC):
                            te.matmul(pl[:], HmT[hc][:, s128(ncp)],
                                      EbarT[hc][:],
                                      start=(hc == 0), stop=(hc == HC - 1))
                        expP = sm.tile([128, 64], f32, tag="expP", bufs=2,
                                       name="expP")
                        se = sm.tile([128, 1], f32, tag="se", bufs=2, name="se")
                        sc.activation(expP[:], pl[:], AF.Exp, accum_out=se[:])
                        invp = sm.tile([128, 1], f32, tag="invp", bufs=2,
                                       name="invp")
                        v.reciprocal(invp[:], se[:])
                        P = sm.tile([128, 64], f32, tag="P", bufs=4, name="P")
                        v.tensor_scalar(P[:], expP[:], invp[:], None, OP.mult)
                        v.tensor_tensor(Mfull[:, s64(ncp)], P[:],
                                        bern[:, s64(ncp)], OP.is_gt)
                        if debug:
                            nc.sync.dma_start(out=dbg["P_dbg"][s128(ncp), :],
                                              in_=P[:])
                            nc.sync.dma_start(out=dbg["M_dbg"][s128(ncp), :],
                                              in_=Mfull[:, s64(ncp)])
                        Pq.append(P)
                    ptp = psT.tile([64, 512], f32, tag="tr", name="ptp")
                    for q in range(4):
                        te.transpose(ptp[:, s128(q)], Pq[q][:], ident[:])
                    v.tensor_copy(PT[:, s512(j)], ptp[:])
                    mtp = psT.tile([64, 512], f32, tag="tr", name="mtp")
                    for q in range(4):
                        te.transpose(mtp[:, s128(q)], Mfull[:, s64(4 * j + q)],
                                     ident[:])
                    v.tensor_copy(MT[:, s512(j)], mtp[:])

                # Gram matrix Gm = M.T @ M (fp32, N=64 outputs)
                gm_ps = psP.tile([64, 64], f32, tag="small", bufs=2, name="gm")
                for ncp in range(NCP):
                    te.matmul(gm_ps[:], Mfull[:, s64(ncp)], Mfull[:, s64(ncp)],
                              start=(ncp == 0), stop=(ncp == NCP - 1))
                Gm = med.tile([64, 64], f32, tag="Gm")
                v.tensor_copy(Gm[:], gm_ps[:])

            # ---- phase 6: KmatT = wk.T @ HmT + wk_b  fp32r ---------------
            if upto >= 6:
                KmatT = [big.tile([128, N], f32r, tag="B", name=f"KmatT{i}")
                         for i in range(HC)]
                for hc in range(HC):
                    for j in range(NCJ):
                        pp = psA.tile([128, 512], f32, tag="mm", name="ppK")
                        for h1c in range(HC):
                            te.matmul(pp[:], wchunk(wkt, h1c, hc),
                                      HmT[h1c][:, s512(j)],
                                      start=(h1c == 0), stop=(h1c == HC - 1))
                        sc.activation(KmatT[hc][:, s512(j)], pp[:], AF.Identity,
                                      bias=wkbc[:, hc:hc + 1], scale=1.0)

            # ---- phase 7: scores -> expS (unnormalized), expST -----------
            if upto >= 7:
                expS = med.tile([64, N], f32, tag="expS")
                pses = []
                for j in range(NCJ):
                    ps_ = psS.tile([64, 512], f32, tag="s64", name="psc")
                    for hc in range(HC):
                        te.matmul(ps_[:], QT[hc][:], KmatT[hc][:, s512(j)],
                                  start=(hc == 0), stop=(hc == HC - 1))
                    pse = med.tile([64, 1], f32, tag=f"pse{j}", name=f"pse{j}")
                    sc.activation(expS[:, s512(j)], ps_[:], AF.Exp,
                                  accum_out=pse[:])
                    pses.append(pse)
                sA = med.tile([64, 1], f32, tag="sA")
                v.tensor_tensor(sA[:], pses[0][:], pses[1][:], OP.add)
                sA2 = med.tile([64, 1], f32, tag="sA2")
                v.tensor_tensor(sA2[:], pses[2][:], pses[3][:], OP.add)
                sA3 = med.tile([64, 1], f32, tag="sA3")
                v.tensor_tensor(sA3[:], sA[:], sA2[:], OP.add)
                invA = med.tile([64, 1], f32, tag="invA")
                v.reciprocal(invA[:], sA3[:])

                expST = med.tile([128, NCP * 64], f32r, tag="expST")
                for j in range(NCJ):
                    pt = psT.tile([128, 256], f32, tag="tr", name="pte")
                    for q in range(4):
                        te.transpose(pt[:, s64(q)],
                                     expS[0:64, s128(4 * j + q)],
                                     ident[0:64, 0:64])
                    v.tensor_copy(expST[:, j * 256:(j + 1) * 256], pt[:])

            # ---- phase 8: Ctemp = (A @ Hm) fp32r -------------------------
            if upto >= 8:
                pc = psS.tile([64, 512], f32, tag="s64", name="pc")
                for ncp in range(NCP):
                    te.matmul(pc[:], expST[:, s64(ncp)],
                              HmB[ncp // 4][:, s512(ncp % 4)],
                              start=(ncp == 0), stop=(ncp == NCP - 1))
                Ctemp = med.tile([64, H], f32, tag="Ctemp")
                v.tensor_scalar(Ctemp[:], pc[:], invA[:], None, OP.mult)
                if debug:
                    nc.sync.dma_start(out=dbg["Ct_dbg"][0:64, :],
                                      in_=Ctemp[:])

            # ---- phase 9: norms via Gram; Ctemp_s; Hupd; C output --------
            if upto >= 9:
                gc_ps = psS.tile([64, 512], f32, tag="s64", name="gc")
                te.matmul(gc_ps[:], Gm[:], Ctemp[:], start=True, stop=True)
                prod = med.tile([64, H], f32, tag="prod")
                v.tensor_tensor(prod[:], Ctemp[:], gc_ps[:], OP.mult)
                n2_ps = psS.tile([1, 512], f32, tag="s64", name="n2")
                te.matmul(n2_ps[:], ones64c[:], prod[:], start=True, stop=True)
                if debug:
                    nroot_d = med.tile([1, H], f32, tag="nroot_d")
                    v.tensor_copy(nroot_d[:], n2_ps[:])
                    nc.sync.dma_start(out=dbg["n2_dbg"][:, :], in_=nroot_d[:])
                nroot = med.tile([1, H], f32, tag="nroot")
                sc.sqrt(nroot[:], n2_ps[:])
                v.tensor_scalar(nroot[:], nroot[:], 1e-12, None, OP.max)
                invn_row = med.tile([1, H], f32, tag="invn_row")
                v.reciprocal(invn_row[:], nroot[:])
                bc_ps = psS.tile([64, 512], f32, tag="s64", name="bc")
                te.matmul(bc_ps[:], ones1_64[:], invn_row[:], start=True,
                          stop=True)
                Ctemp_s = med.tile([64, H], f32r, tag="Ctemp_s")
                v.tensor_tensor(Ctemp_s[:], Ctemp[:], bc_ps[:], OP.mult)

                # H_upd (in place on HmT): HmT += Ctemp_s.T @ MT
                for hc in range(HC):
                    for j in range(NCJ):
                        pp = psA.tile([128, 512], f32, tag="mm", name="ppCp")
                        te.matmul(pp[:], Ctemp_s[:, s128(hc)], MT[:, s512(j)],
                                  start=True, stop=True)
                        v.tensor_tensor(HmT[hc][:, s512(j)], pp[:],
                                        HmT[hc][:, s512(j)], OP.add)

                # C output (already normalized): MT.T @ Ctemp_s, per-chunk DMA
                for j in range(NCJ):
                    cb = stg.tile([128, N], f32, tag="xq", bufs=2, name=f"cb{j}")
                    for q in range(4):
                        ncp = 4 * j + q
                        pp = psA.tile([128, 512], f32, tag="mm", name="ppCo")
                        te.matmul(pp[:], MT[:, s128(ncp)], Ctemp_s[:],
                                  start=True, stop=True)
                        sc.activation(cb[:, s512(q)], pp[:], AF.Copy)
                        nc.sync.dma_start(out=Co_d[s128(ncp), :],
                                          in_=cb[:, s512(q)])

            # ---- phase 10/11: G = H_upd * (E.T @ PT), in place -----------
            if upto >= 10:
                for hc in range(HC):
                    for j in range(NCJ):
                        pth = psA.tile([128, 512], f32, tag="mm", name="ppTh")
                        te.matmul(pth[:], E_r[:, s128(hc)], PT[:, s512(j)],
                                  start=True, stop=True)
                        v.tensor_tensor(HmT[hc][:, s512(j)], pth[:],
                                        HmT[hc][:, s512(j)], OP.mult)

            # ---- phase 12: Y = G @ wout (+ wout_b in-GEMM), per-chunk DMA
            if upto >= 11:
                for j in range(NCJ):
                    yb = stg.tile([128, N], f32, tag="xq", bufs=2, name=f"yb{j}")
                    for q in range(4):
                        ncp = 4 * j + q
                        pp = psA.tile([128, 512], f32, tag="mm", name="ppY")
                        # bias term: ones128r.T @ wobrow_r broadcasts wout_b
                        te.matmul(pp[:], ones128r[:], wobrow_r[:],
                                  start=True, stop=False)
                        for hc in range(HC):
                            te.matmul(pp[:], HmT[hc][:, s128(ncp)],
                                      wot[:, s512(hc)],
                                      start=False, stop=(hc == HC - 1))
                        sc.activation(yb[:, s512(q)], pp[:], AF.Copy)
                        nc.sync.dma_start(out=Y_d[s128(ncp), :],
                                          in_=yb[:, s512(q)])

    nc.finalize()
    return nc


def _get_nc():
    if "nc" not in _CACHE:
        _CACHE["nc"] = build_nc()
    return _CACHE["nc"]


def kernel(**inputs):
    from concourse.bass_utils import run_bass_kernel_spmd

    nc = _get_nc()
    arr = {k: np.ascontiguousarray(np.asarray(v, dtype=np.float32))
           for k, v in inputs.items()}
    shared = {k: arr[k] for k in
              ("cluster_embeddings", "mlp_w1", "mlp_b1", "mlp_w2", "mlp_b2",
               "wq", "wq_b", "wk", "wk_b", "wout", "wout_b")}
    in_maps = [dict(x=arr["x"][b], bern_u=arr["bern_u"][b], **shared)
               for b in range(B)]
    res = run_bass_kernel_spmd(nc, in_maps, list(range(B))).results
    Y = np.stack([res[b]["Y"] for b in range(B)])
    Co = np.stack([res[b]["C_out"] for b in range(B)])
    return (Y, Co)


if __name__ == "__main__":
    import os
    os.environ.setdefault("JAX_PLATFORMS", "cpu")
    from concourse.timeline_sim import TimelineSim

    nc = build_nc()
    ts = TimelineSim(nc, trace=False)
    print("TimelineSim:", ts.simulate(), "ns")


# revision 29
# speedup vs baseline: 1.1923x; 1.1923x over previous
"""Trainium2 Bass kernel for nn_CCM_73985106641118 (vq_codebook).

Data-parallel across the batch dim: core b processes batch b (8 cores, B=8).

Design (v3):
- The precision-critical x -> H1 -> Hm GEMM chain runs in fp32r
  (1 cyc/row on the PE at N=512, 4x faster than fp32). Everything at or
  past the Hm quantization point runs in bf16: logits/Kmat/scores read a
  bf16 Hm, the mask/attention/theta paths are bf16 throughout. The
  bern_u < P comparison tolerates the resulting ~5e-4 logit error
  (a handful of mask flips per core vs the 2e-2 gate).
- Weights are staged and rounded to fp32r/bf16 by the otherwise-idle
  gpsimd engine; 5 weights rotate through 3 slots.
- Hm is produced feature-major first (HmT) so the b2 bias rides the
  PSUM drain; the token-major HmB comes from bf16 PE transposes (1
  cyc/row) batched four-per-PSUM-bank per drain.
- C_pre l2-norms come from the Gram identity n2[h] = c^T (M^T M) c, so
  C_pre is never materialized unnormalized: Ctemp is pre-scaled once
  and both CpreT (H update) and the C output emerge normalized from
  the PE. wout_b is folded into the Y GEMM as a rank-1 ones term.
- PSUM drains are fused with the adjacent elementwise op and split
  across the scalar and vector engines.
"""

import numpy as np

import concourse.bacc as bacc
import concourse.mybir as mybir
from concourse.masks import make_identity
from concourse.tile import TileContext

f32 = mybir.dt.float32
f32r = mybir.dt.float32r
bf16 = mybir.dt.bfloat16
AX = mybir.AxisListType.X
OP = mybir.AluOpType
AF = mybir.ActivationFunctionType

B, N, C, H, K = 8, 2048, 512, 512, 64
NCP = N // 128   # 16 token chunks of 128
NCJ = N // 512   # 4 token chunks of 512
HC = H // 128    # 4 feature chunks of 128
SCALE = 1.0 / np.sqrt(np.float32(H))

_CACHE = {}


def s128(i):
    return slice(i * 128, (i + 1) * 128)


def s512(i):
    return slice(i * 512, (i + 1) * 512)


def s64(i):
    return slice(i * 64, (i + 1) * 64)


def build_nc(debug=False, upto=99):
    nc = bacc.Bacc("TRN2", target_bir_lowering=False, debug=False, num_devices=8)

    x_d = nc.dram_tensor("x", [N, C], f32, kind="ExternalInput").ap()
    bu_d = nc.dram_tensor("bern_u", [N, K], f32, kind="ExternalInput").ap()
    E_d = nc.dram_tensor("cluster_embeddings", [K, H], f32, kind="ExternalInput").ap()
    w1_d = nc.dram_tensor("mlp_w1", [C, H], f32, kind="ExternalInput").ap()
    b1_d = nc.dram_tensor("mlp_b1", [H], f32, kind="ExternalInput").ap()
    w2_d = nc.dram_tensor("mlp_w2", [H, H], f32, kind="ExternalInput").ap()
    b2_d = nc.dram_tensor("mlp_b2", [H], f32, kind="ExternalInput").ap()
    wq_d = nc.dram_tensor("wq", [H, H], f32, kind="ExternalInput").ap()
    wqb_d = nc.dram_tensor("wq_b", [H], f32, kind="ExternalInput").ap()
    wk_d = nc.dram_tensor("wk", [H, H], f32, kind="ExternalInput").ap()
    wkb_d = nc.dram_tensor("wk_b", [H], f32, kind="ExternalInput").ap()
    wout_d = nc.dram_tensor("wout", [H, C], f32, kind="ExternalInput").ap()
    woutb_d = nc.dram_tensor("wout_b", [C], f32, kind="ExternalInput").ap()
    Y_d = nc.dram_tensor("Y", [N, C], f32, kind="ExternalOutput").ap()
    Co_d = nc.dram_tensor("C_out", [N, H], f32, kind="ExternalOutput").ap()
    dbg = {}
    if debug:
        for nm, shp in [("P_dbg", [N, K]), ("M_dbg", [N, K]), ("HmT_dbg", [H, N]),
                        ("Ct_dbg", [64, H]), ("n2_dbg", [1, H])]:
            dbg[nm] = nc.dram_tensor(nm, shp, f32, kind="ExternalOutput").ap()

    with TileContext(nc) as tc:
        with (
            tc.tile_pool(name="big", bufs=4) as big,
            tc.tile_pool(name="med", bufs=1) as med,
            tc.tile_pool(name="sm", bufs=2) as sm,
            tc.tile_pool(name="stg", bufs=2) as stg,
            tc.tile_pool(name="psA", bufs=4, space="PSUM") as psA,
            tc.tile_pool(name="psT", bufs=2, space="PSUM") as psT,
            tc.tile_pool(name="psP", bufs=1, space="PSUM") as psP,
            tc.tile_pool(name="psS", bufs=1, space="PSUM") as psS,
        ):
            v = nc.vector
            sc = nc.scalar
            te = nc.tensor
            gp = nc.gpsimd

            # ---- x DMAs first on the (multi-engine) sync queue -----------
            xqs = []
            for j in range(NCJ):
                xq = stg.tile([128, N], f32, tag="xq", bufs=2, name=f"xq{j}")
                nc.sync.dma_start(
                    out=xq[:].rearrange("p (q c) -> p q c", q=4),
                    in_=x_d[s512(j), :].rearrange("(q p) c -> p q c", p=128))
                xqs.append(xq)

            # ---- small inputs on the Activation HWDGE queue --------------
            E_f = med.tile([64, H], f32, tag="E")
            nc.scalar.dma_start(out=E_f[:], in_=E_d[:, :])

            def bias_cols(dram, tag):
                t = med.tile([128, HC], f32, tag=tag, name=tag)
                nc.scalar.dma_start(out=t[:],
                                    in_=dram.rearrange("(j p) -> p j", p=128))
                return t

            b1c = bias_cols(b1_d, "b1c")
            b2c = bias_cols(b2_d, "b2c")
            wkbc = bias_cols(wkb_d, "wkbc")
            wqbc0 = bias_cols(wqb_d, "wqbc0")
            wobrow = med.tile([1, C], f32, tag="wobrow")
            nc.scalar.dma_start(out=wobrow[:],
                                in_=woutb_d.rearrange("(o a) -> o a", o=1))
            bern = med.tile([128, NCP * K], f32, tag="bern")
            nc.scalar.dma_start(out=bern[:].rearrange("p (q k) -> p q k", q=16),
                                in_=bu_d.rearrange("(q p) k -> p q k", p=128))

            # ---- constants -----------------------------------------------
            ident = med.tile([128, 128], f32, tag="ident")
            make_identity(nc, ident[:])
            identb = med.tile([128, 128], bf16, tag="identb")
            make_identity(nc, identb[:])

            wqbc = med.tile([128, HC], f32, tag="wqbc")
            v.tensor_scalar(wqbc[:], wqbc0[:], float(SCALE), None, OP.mult)
            ones128 = med.tile([1, 128], f32, tag="ones")
            gp.memset(ones128[:], 1.0)
            ones64c = med.tile([64, 1], f32, tag="ones64c")
            gp.memset(ones64c[:], 1.0)
            ones1_64 = med.tile([1, 64], f32, tag="ones1_64")
            gp.memset(ones1_64[:], 1.0)
            ones128b = med.tile([1, 128], bf16, tag="onesb")
            gp.tensor_copy(ones128b[:], ones128[:])
            wobrow_b = med.tile([1, C], bf16, tag="wobrow_b")
            gp.tensor_copy(wobrow_b[:], wobrow[:])

            def load_w(dram, tag, dt):
                # 5 weights rotate through 3 slots: w1/w2 are dead after
                # phases 2/3, so wq/wout copies wait for those readers
                # (loaded after the phase-2 trace to keep the gpsimd queue
                # deadlock-free). Rounding to fp32r/bf16 happens on gpsimd.
                t = med.tile([128, HC * 512], dt, tag="w", bufs=3, name=tag)
                for cc in range(HC):
                    st = sm.tile([128, 512], f32, tag="wst", bufs=2,
                                 name=f"wst_{tag}{cc}")
                    nc.sync.dma_start(out=st[:], in_=dram[s128(cc), :])
                    gp.tensor_copy(t[:, s512(cc)], st[:])
                return t

            w1t = load_w(w1_d, "w1t", f32r)
            w2t = load_w(w2_d, "w2t", f32r)
            wkt = load_w(wk_d, "wkt", bf16)

            def wchunk(t, cc, blk):
                return t[:, cc * 512 + blk * 128: cc * 512 + (blk + 1) * 128]

            # E variants: bf16 token-major, bf16 feature-major (EbarT/ET)
            E_rb = med.tile([64, H], bf16, tag="Erb")
            gp.tensor_copy(E_rb[:], E_f[:])

            esq = sm.tile([128, C], f32, tag="xsq", bufs=1, name="esq")[0:64, :]
            ensq = med.tile([64, 1], f32, tag="ensq")
            sc.activation(esq[:], E_f[:], AF.Square, accum_out=ensq[:])
            enrm = med.tile([64, 1], f32, tag="enrm")
            sc.sqrt(enrm[:], ensq[:])
            einv = med.tile([64, 1], f32, tag="einv")
            v.reciprocal(einv[:], enrm[:])
            Ebar = med.tile([64, H], bf16, tag="Ebar")
            v.tensor_scalar(Ebar[:], E_f[:], einv[:], None, OP.mult)

            EbarT, ET = [], []
            for hc in range(HC):
                pt = psP.tile([128, 64], bf16, tag="small", name="ptE")
                te.transpose(pt[:], Ebar[:, s128(hc)], identb[0:64, 0:64])
                t = med.tile([128, 64], bf16, tag=f"ebt{hc}", name=f"ebt{hc}")
                v.tensor_copy(t[:], pt[:])
                EbarT.append(t)
                pt2 = psP.tile([128, 64], bf16, tag="small", name="ptE2")
                te.transpose(pt2[:], E_rb[:, s128(hc)], identb[0:64, 0:64])
                t2 = med.tile([128, 64], bf16, tag=f"et{hc}", name=f"et{hc}")
                v.tensor_copy(t2[:], pt2[:])
                ET.append(t2)

            # ---- phase 1: xn = l2norm(x) rows (in place), xnT fp32r ------
            if upto >= 1:
                xnT = [big.tile([128, N], f32r, tag="A", name=f"xnT{i}")
                       for i in range(HC)]
                for j in range(NCJ):
                    xq = xqs[j]
                    ssq = sm.tile([128, 4], f32, tag="ssq", bufs=2, name="ssq")
                    xsq = sm.tile([128, C], f32, tag="xsq", bufs=1, name="xsq")
                    for q in range(4):
                        sc.activation(xsq[:], xq[:, s512(q)], AF.Square,
                                      accum_out=ssq[:, q:q + 1])
                    nrm = sm.tile([128, 4], f32, tag="nrm", bufs=2, name="nrm")
                    sc.sqrt(nrm[:], ssq[:])
                    nrm2 = sm.tile([128, 4], f32, tag="nrm2", bufs=2, name="nrm2")
                    v.tensor_scalar(nrm2[:], nrm[:], 1e-12, None, OP.max)
                    inv = sm.tile([128, 4], f32, tag="inv", bufs=2, name="inv")
                    v.reciprocal(inv[:], nrm2[:])
                    for q in range(4):
                        v.tensor_scalar(xq[:, s512(q)], xq[:, s512(q)],
                                        inv[:, q:q + 1], None, OP.mult)
                    for cc in range(HC):
                        pt = psT.tile([128, 512], f32, tag="tr", name="ptx")
                        for q in range(4):
                            te.transpose(pt[:, s128(q)],
                                         xq[:, q * 512 + cc * 128:
                                            q * 512 + (cc + 1) * 128],
                                         ident[:])
                        v.tensor_copy(xnT[cc][:, s512(j)], pt[:])

            # ---- phase 2: H1T = relu(w1.T @ xnT + b1)  fp32r -------------
            if upto >= 2:
                H1T = [big.tile([128, N], f32r, tag="B", name=f"H1T{i}")
                       for i in range(HC)]
                for j in range(NCJ):
                    for h1c in range(HC):
                        pp = psA.tile([128, 512], f32, tag="mm", name="ppH1")
                        for cc in range(HC):
                            te.matmul(pp[:], wchunk(w1t, cc, h1c),
                                      xnT[cc][:, s512(j)],
                                      start=(cc == 0), stop=(cc == HC - 1))
                        sc.activation(H1T[h1c][:, s512(j)], pp[:], AF.Relu,
                                      bias=b1c[:, h1c:h1c + 1], scale=1.0)

            # wq/wout reuse the w1/w2 slots; loading them here (after the
            # phase-2 trace) keeps the gpsimd queue deadlock-free.
            wqt = load_w(wq_d, "wqt", bf16)
            wot = load_w(wout_d, "wot", bf16)

            # ---- QT = (wq.T @ ET + wq_b) * scale  (bf16) -----------------
            QT = []
            for hc in range(HC):
                pq = psP.tile([128, 64], f32, tag="small", name="pq")
                for cc in range(HC):
                    te.matmul(pq[:], wchunk(wqt, cc, hc), ET[cc][:],
                              start=(cc == 0), stop=(cc == HC - 1))
                t = med.tile([128, 64], bf16, tag=f"qt{hc}", name=f"qt{hc}")
                sc.activation(t[:], pq[:], AF.Identity,
                              bias=wqbc[:, hc:hc + 1], scale=float(SCALE))
                QT.append(t)

            # ---- phase 3: HmT = w2.T @ H1T + b2 (feature-major, bf16) ----
            if upto >= 3:
                HmT = [big.tile([128, N], bf16, tag="C", name=f"HmT{i}")
                       for i in range(HC)]
                for j in range(NCJ):
                    for hc in range(HC):
                        pp = psA.tile([128, 512], f32, tag="mm", name="ppHm")
                        for h1c in range(HC):
                            te.matmul(pp[:], wchunk(w2t, h1c, hc),
                                      H1T[h1c][:, s512(j)],
                                      start=(h1c == 0), stop=(h1c == HC - 1))
                        v.tensor_scalar(HmT[hc][:, s512(j)], pp[:],
                                        b2c[:, hc:hc + 1], None, OP.add)
                        if debug:
                            hmd = sm.tile([128, 512], f32, tag="hmd", bufs=2,
                                          name="hmd")
                            v.tensor_copy(hmd[:], HmT[hc][:, s512(j)])
                            nc.sync.dma_start(
                                out=dbg["HmT_dbg"][s128(hc), s512(j)],
                                in_=hmd[:])

            # ---- phase 4: HmB token-major via bf16 PE transposes ---------
            if upto >= 4:
                HmB = [big.tile([128, N], bf16, tag="A", name=f"HmB{i}")
                       for i in range(NCJ)]
                for j in range(NCJ):
                    for q in range(4):
                        ncp = 4 * j + q
                        pt = psT.tile([128, 512], bf16, tag="tr", name="ptm")
                        for hc in range(HC):
                            te.transpose(pt[:, s128(hc)],
                                         HmT[hc][:, s128(ncp)],
                                         identb[:])
                        v.tensor_copy(HmB[j][:, s512(q)], pt[:])

            # ---- phase 5: logits -> P -> M; PT / MT (bf16) ---------------
            if upto >= 5:
                PT = med.tile([64, N], bf16, tag="PT")
                MT = med.tile([64, N], bf16, tag="MT")
                Mfull = med.tile([128, NCP * K], bf16, tag="Mfull")
                for j in range(NCJ):
                    Pq = []
                    for q in range(4):
                        ncp = 4 * j + q
                        pl = psP.tile([128, 64], f32, tag="small", name="pl")
                        for hc in range(HC):
                            te.matmul(pl[:], HmT[hc][:, s128(ncp)],
                                      EbarT[hc][:],
                                      start=(hc == 0), stop=(hc == HC - 1))
                        expP = sm.tile([128, 64], f32, tag="expP", bufs=2,
                                       name="expP")
                        se = sm.tile([128, 1], f32, tag="se", bufs=2, name="se")
                        sc.activation(expP[:], pl[:], AF.Exp, accum_out=se[:])
                        invp = sm.tile([128, 1], f32, tag="invp", bufs=2,
                                       name="invp")
                        v.reciprocal(invp[:], se[:])
                        P = sm.tile([128, 64], bf16, tag="P", bufs=4, name="P")
                        v.tensor_scalar(P[:], expP[:], invp[:], None, OP.mult)
                        # M = (bern * rowsum < expP)  <=>  bern < P, computed
                        # in fp32 so bf16 P rounding can't flip the mask
                        v.scalar_tensor_tensor(Mfull[:, s64(ncp)],
                                               bern[:, s64(ncp)], se[:],
                                               expP[:], OP.mult, OP.is_lt)
                        Pq.append(P)
                    ptp = psT.tile([64, 512], bf16, tag="tr", name="ptp")
                    for q in range(4):
                        te.transpose(ptp[:, s128(q)], Pq[q][:], identb[:])
                    v.tensor_copy(PT[:, s512(j)], ptp[:])
                    mtp = psT.tile([64, 512], bf16, tag="tr", name="mtp")
                    for q in range(4):
                        te.transpose(mtp[:, s128(q)], Mfull[:, s64(4 * j + q)],
                                     identb[:])
                    v.tensor_copy(MT[:, s512(j)], mtp[:])

                if debug:
                    for ncp in range(NCP):
                        pd = sm.tile([128, 64], f32, tag="pd", bufs=2, name="pd")
                        v.tensor_copy(pd[:], Mfull[:, s64(ncp)])
                        nc.sync.dma_start(out=dbg["M_dbg"][s128(ncp), :],
                                          in_=pd[:])

                # Gram matrix Gm = M.T @ M (bf16 operands, exact 0/1)
                gm_ps = psP.tile([64, 64], f32, tag="small", name="gm")
                for ncp in range(NCP):
                    te.matmul(gm_ps[:], Mfull[:, s64(ncp)], Mfull[:, s64(ncp)],
                              start=(ncp == 0), stop=(ncp == NCP - 1))
                Gm = med.tile([64, 64], f32, tag="Gm")
                v.tensor_copy(Gm[:], gm_ps[:])

            # ---- phase 6: KmatT = wk.T @ HmT + wk_b  (bf16) --------------
            if upto >= 6:
                KmatT = [big.tile([128, N], bf16, tag="B", name=f"KmatT{i}")
                         for i in range(HC)]
                for hc in range(HC):
                    for j in range(NCJ):
                        pp = psA.tile([128, 512], f32, tag="mm", name="ppK")
                        for h1c in range(HC):
                            te.matmul(pp[:], wchunk(wkt, h1c, hc),
                                      HmT[h1c][:, s512(j)],
                                      start=(h1c == 0), stop=(h1c == HC - 1))
                        sc.activation(KmatT[hc][:, s512(j)], pp[:], AF.Identity,
                                      bias=wkbc[:, hc:hc + 1], scale=1.0)

            # ---- phase 7: scores -> expS (unnormalized), expST -----------
            if upto >= 7:
                expS = med.tile([64, N], bf16, tag="expS")
                pses = []
                for j in range(NCJ):
                    ps_ = psS.tile([64, 512], f32, tag="s64", name="psc")
                    for hc in range(HC):
                        te.matmul(ps_[:], QT[hc][:], KmatT[hc][:, s512(j)],
                                  start=(hc == 0), stop=(hc == HC - 1))
                    pse = med.tile([64, 1], f32, tag=f"pse{j}", name=f"pse{j}")
                    sc.activation(expS[:, s512(j)], ps_[:], AF.Exp,
                                  accum_out=pse[:])
                    pses.append(pse)
                sA = med.tile([64, 1], f32, tag="sA")
                v.tensor_tensor(sA[:], pses[0][:], pses[1][:], OP.add)
                sA2 = med.tile([64, 1], f32, tag="sA2")
                v.tensor_tensor(sA2[:], pses[2][:], pses[3][:], OP.add)
                sA3 = med.tile([64, 1], f32, tag="sA3")
                v.tensor_tensor(sA3[:], sA[:], sA2[:], OP.add)
                invA = med.tile([64, 1], f32, tag="invA")
                v.reciprocal(invA[:], sA3[:])

                expST = med.tile([128, NCP * 64], bf16, tag="expST")
                for j in range(NCJ):
                    pt = psT.tile([128, 256], bf16, tag="tr", name="pte")
                    for q in range(4):
                        te.transpose(pt[:, s64(q)],
                                     expS[0:64, s128(4 * j + q)],
                                     identb[0:64, 0:64])
                    v.tensor_copy(expST[:, j * 256:(j + 1) * 256], pt[:])

            # ---- phase 8: Ctemp = (A @ Hm) -------------------------------
            if upto >= 8:
                pc = psS.tile([64, 512], f32, tag="s64", name="pc")
                for ncp in range(NCP):
                    te.matmul(pc[:], expST[:, s64(ncp)],
                              HmB[ncp // 4][:, s512(ncp % 4)],
                              start=(ncp == 0), stop=(ncp == NCP - 1))
                Ctemp = med.tile([64, H], f32, tag="Ctemp")
                v.tensor_scalar(Ctemp[:], pc[:], invA[:], None, OP.mult)
                if debug:
                    nc.sync.dma_start(out=dbg["Ct_dbg"][0:64, :], in_=Ctemp[:])

            # ---- phase 9: norms via Gram; Ctemp_s; Hupd; C output --------
            if upto >= 9:
                gc_ps = psS.tile([64, 512], f32, tag="s64", name="gc")
                te.matmul(gc_ps[:], Gm[:], Ctemp[:], start=True, stop=True)
                prod = med.tile([64, H], f32, tag="prod")
                v.tensor_tensor(prod[:], Ctemp[:], gc_ps[:], OP.mult)
                n2_ps = psS.tile([1, 512], f32, tag="s64", name="n2")
                te.matmul(n2_ps[:], ones64c[:], prod[:], start=True, stop=True)
                if debug:
                    nroot_d = med.tile([1, H], f32, tag="nroot_d")
                    v.tensor_copy(nroot_d[:], n2_ps[:])
                    nc.sync.dma_start(out=dbg["n2_dbg"][:, :], in_=nroot_d[:])
                nroot = med.tile([1, H], f32, tag="nroot")
                sc.sqrt(nroot[:], n2_ps[:])
                v.tensor_scalar(nroot[:], nroot[:], 1e-12, None, OP.max)
                invn_row = med.tile([1, H], f32, tag="invn_row")
                v.reciprocal(invn_row[:], nroot[:])
                bc_ps = psS.tile([64, 512], f32, tag="s64", name="bc")
                te.matmul(bc_ps[:], ones1_64[:], invn_row[:], start=True,
                          stop=True)
                Ctemp_s = med.tile([64, H], bf16, tag="Ctemp_s")
                v.tensor_tensor(Ctemp_s[:], Ctemp[:], bc_ps[:], OP.mult)

                # H_upd (in place on HmT): HmT += Ctemp_s.T @ MT
                for hc in range(HC):
                    for j in range(NCJ):
                        pp = psA.tile([128, 512], f32, tag="mm", name="ppCp")
                        te.matmul(pp[:], Ctemp_s[:, s128(hc)], MT[:, s512(j)],
                                  start=True, stop=True)
                        v.tensor_tensor(HmT[hc][:, s512(j)], pp[:],
                                        HmT[hc][:, s512(j)], OP.add)

                # C output (already normalized): MT.T @ Ctemp_s, per-chunk DMA
                for j in range(NCJ):
                    cb = stg.tile([128, N], f32, tag="xq", bufs=2, name=f"cb{j}")
                    for q in range(4):
                        ncp = 4 * j + q
                        pp = psA.tile([128, 512], f32, tag="mm", name="ppCo")
                        te.matmul(pp[:], MT[:, s128(ncp)], Ctemp_s[:],
                                  start=True, stop=True)
                        sc.activation(cb[:, s512(q)], pp[:], AF.Copy)
                        nc.sync.dma_start(out=Co_d[s128(ncp), :],
                                          in_=cb[:, s512(q)])

            # ---- phase 10/11: G = H_upd * (E.T @ PT), in place -----------
            if upto >= 10:
                for hc in range(HC):
                    for j in range(NCJ):
                        pth = psA.tile([128, 512], f32, tag="mm", name="ppTh")
                        te.matmul(pth[:], E_rb[:, s128(hc)], PT[:, s512(j)],
                                  start=True, stop=True)
                        v.tensor_tensor(HmT[hc][:, s512(j)], pth[:],
                                        HmT[hc][:, s512(j)], OP.mult)

            # ---- phase 12: Y = G @ wout (+ wout_b in-GEMM), per-chunk DMA
            if upto >= 11:
                for j in range(NCJ):
                    yb = stg.tile([128, N], f32, tag="xq", bufs=2, name=f"yb{j}")
                    for q in range(4):
                        ncp = 4 * j + q
                        pp = psA.tile([128, 512], f32, tag="mm", name="ppY")
                        te.matmul(pp[:], ones128b[:], wobrow_b[:],
                                  start=True, stop=False)
                        for hc in range(HC):
                            te.matmul(pp[:], HmT[hc][:, s128(ncp)],
                                      wot[:, s512(hc)],
                                      start=False, stop=(hc == HC - 1))
                        if q % 2 == 0:
                            sc.activation(yb[:, s512(q)], pp[:], AF.Copy)
                        else:
                            v.tensor_copy(yb[:, s512(q)], pp[:])
                        nc.sync.dma_start(out=Y_d[s128(ncp), :],
                                          in_=yb[:, s512(q)])

    nc.finalize()
    return nc


def _get_nc():
    if "nc" not in _CACHE:
        _CACHE["nc"] = build_nc()
    return _CACHE["nc"]


def kernel(**inputs):
    from concourse.bass_utils import run_bass_kernel_spmd

    nc = _get_nc()
    arr = {k: np.ascontiguousarray(np.asarray(v, dtype=np.float32))
           for k, v in inputs.items()}
    shared = {k: arr[k] for k in
              ("cluster_embeddings", "mlp_w1", "mlp_b1", "mlp_w2", "mlp_b2",
               "wq", "wq_b", "wk", "wk_b", "wout", "wout_b")}
    in_maps = [dict(x=arr["x"][b], bern_u=arr["bern_u"][b], **shared)
               for b in range(B)]
    res = run_bass_kernel_spmd(nc, in_maps, list(range(B))).results
    Y = np.stack([res[b]["Y"] for b in range(B)])
    Co = np.stack([res[b]["C_out"] for b in range(B)])
    return (Y, Co)


if __name__ == "__main__":
    import os
    os.environ.setdefault("JAX_PLATFORMS", "cpu")
    from concourse.timeline_sim import TimelineSim

    nc = build_nc()
    ts = TimelineSim(nc, trace=False)
    print("TimelineSim:", ts.simulate(), "ns")
